# revision 8
# baseline (speedup 1.0000x reference)
"""GPT2 non-residual attention on 8 TRN2 NeuronCores (Bass/Tile).

Sharding: rows of (B*S) split 8 ways — core c owns batch b=c//2 and four
interleaved 128-row s-blocks chosen so causal work is balanced across the
two cores of a batch (parity 0 -> blocks {0,3,4,7}, parity 1 -> {1,2,5,6};
both have total causal extent 18 tiles, padded to a uniform static
schedule of EXT=(2,4,6,8) tiles/slot with data-side masks).

All matmuls run in bf16 (rel-err gate is 2e-2; bf16 lands ~1e-3).
Attention uses transposed-score layout: scores^T [t,128s] tiles on PSUM,
exp on ScalarE (scale=1/8 fused, no max pass — scores are provably small),
AV as lhsT=[tV|ones] so the softmax denominator falls out of the matmul,
self-term (q.k diagonal) via ones-matmul partition reduction, per-head
normalization with K=1 broadcast matmuls, c_proj directly from the
transposed attention output.
"""

import numpy as np
import ml_dtypes

import concourse.bass as bass
import concourse.mybir as mybir
import concourse.tile as tile
from concourse import bass_utils

BF16 = mybir.dt.bfloat16
F32 = mybir.dt.float32
AF = mybir.ActivationFunctionType


def _legalize_multi_waits(bir_json):
    """This walrus build encodes at most ONE sem-wait per instruction.
    Split instructions with N>1 waits: hoist N-1 waits onto standalone
    EventSemaphore instructions inserted just before (same engine, same
    program position => identical semantics)."""
    import orjson
    d = orjson.loads(bir_json)
    ctr = [0]
    changed = False
    for fn in d.get("functions", []):
        for blk in fn.get("blocks", []):
            out = []
            for inst in blk.get("instructions", []):
                si = inst.get("sync_info")
                waits = (si or {}).get("on_wait") or []
                if len(waits) > 1:
                    changed = True
                    for w in waits[:-1]:
                        ctr[0] += 1
                        ev = {
                            "engine": inst["engine"],
                            "ins": [],
                            "outs": [],
                            "name": f"WSPLIT-{ctr[0]}",
                            "opcode": "EventSemaphore",
                            "sync_info": {"on_update": [], "on_wait": [w]},
                        }
                        if "debug" in inst:
                            ev["debug"] = inst["debug"]
                        out.append(ev)
                    si["on_wait"] = [waits[-1]]
                out.append(inst)
            blk["instructions"] = out
    return orjson.dumps(d) if changed else bir_json


def _install_wait_legalizer():
    if getattr(bass_utils, "_wait_legalizer_installed", False):
        return
    import concourse.bass2jax as bass2jax
    orig = bass_utils.compile_bir_kernel

    def patched(bir_json, tmpdir, neff_name="file.neff"):
        return orig(_legalize_multi_waits(bir_json), tmpdir, neff_name)

    bass_utils.compile_bir_kernel = patched
    bass2jax.compile_bir_kernel = patched
    bass_utils._wait_legalizer_installed = True


_install_wait_legalizer()

B, S, E = 4, 1024, 1024
H, DH = 16, 64
PL = 64
NCORES = 8
SLOTS = 4
EXT = (2, 4, 6, 8)                      # padded t-tile extents (uniform)
BLOCKS = ((0, 3, 4, 7), (1, 2, 5, 6))   # parity -> s-block ids
SCALE = 0.125                            # 1/sqrt(DH)
BFNP = ml_dtypes.bfloat16

_CACHE: dict = {}


def _build(qb0=True, pjb0=True):
    nc = bass.Bass("TRN2", target_bir_lowering=False, debug=False)

    def din(name, shape, dtype=BF16):
        return nc.dram_tensor(name, list(shape), dtype, kind="ExternalInput").ap()

    hsT = din("hsT", (128, 8, 512))        # (p, kc, s): hs_rows[s, kc*128+p]
    wq = din("wq", (128, 8, 8, 128))       # (p, kc, ct, c)
    wk = din("wk", (128, 8, 8, 128))
    wv = din("wv", (128, 8, 8, 128))
    bqkv = din("bqkv", (128, 24), F32)     # bias^T col-tiles: q 0-7, k 8-15, v 16-23
    tkt = din("tkt", (128, 8, 1024))       # ((h%2)*64+d, hp, t)
    tva = din("tva", (128, 16, 8, 65))     # (t_loc, h, j, d|ones)
    pkt = din("pkt", (128, 8, 64))         # ((h%2)*64+d, hp, p)
    pva = din("pva", (64, 16, 65))         # (p, h, d|ones)
    mks = din("mks", (128, 4, 2, 128))     # (t_loc, slot, m, s_loc)
    pmt = din("pmt", (64, 512))            # prompt mask^T (p, s)
    pw = din("pw", (128, 8, 1024))         # (p, fc, e)
    pb = din("pb", (128, 1024), F32)       # c_proj bias broadcast over partitions
    out = nc.dram_tensor("out", [512, 1024], F32, kind="ExternalOutput").ap()

    with tile.TileContext(nc) as tc:
        with (
            tc.tile_pool(name="const", bufs=1) as cp,
            tc.tile_pool(name="work", bufs=3) as wp,
            tc.tile_pool(name="psum", bufs=8, space="PSUM") as pp,
        ):
            def ctile(name, shape, dtype=BF16):
                t = cp.tile(list(shape), dtype, tag=name, name=name)
                return t

            sb_hsT = ctile("sb_hsT", (128, 8, 512))
            sb_wq = ctile("sb_wq", (128, 8, 8, 128))
            sb_wk = ctile("sb_wk", (128, 8, 8, 128))
            sb_wv = ctile("sb_wv", (128, 8, 8, 128))
            sb_bqkv = ctile("sb_bqkv", (128, 24), F32)
            sb_tkt = ctile("sb_tkt", (128, 8, 1024))
            sb_tva = ctile("sb_tva", (128, 16, 8, 65))
            sb_pkt = ctile("sb_pkt", (128, 8, 64))
            sb_pva = ctile("sb_pva", (64, 16, 65))
            sb_mks = ctile("sb_mks", (128, 4, 2, 128))
            sb_pmt = ctile("sb_pmt", (64, 512))
            sb_pw = ctile("sb_pw", (128, 8, 1024))
            sb_pb = ctile("sb_pb", (128, 1024), F32)

            sb_qT = ctile("sb_qT", (128, 8, 512))
            sb_kT = ctile("sb_kT", (128, 8, 512))
            sb_vta = [ctile(f"sb_vta{h}", (65, 512)) for h in range(H)]
            sb_attnT = [ctile(f"sb_attnT{hp}", (128, 512)) for hp in range(8)]
            ones_col = ctile("ones_col", (128, 1))          # bf16 ones (lhsT self)
            ones_row = ctile("ones_row", (1, 128))          # bf16 ones (K=1 bcast)
            ones_rowf = ctile("ones_rowf", (1, 128), F32)   # f32 ones (K=1 bcast)

            for ap_in, sb in (
                (hsT, sb_hsT), (wq, sb_wq), (wk, sb_wk), (wv, sb_wv),
                (bqkv, sb_bqkv), (tkt, sb_tkt), (tva, sb_tva), (pkt, sb_pkt),
                (pva, sb_pva), (mks, sb_mks), (pmt, sb_pmt), (pw, sb_pw),
                (pb, sb_pb),
            ):
                nc.sync.dma_start(out=sb[:], in_=ap_in[:])

            nc.vector.memset(ones_col[:], 1.0)
            nc.vector.memset(ones_row[:], 1.0)
            nc.vector.memset(ones_rowf[:], 1.0)
            for h in range(H):
                nc.vector.memset(sb_vta[h][64:65, :], 1.0)

            def ps_tile(name):
                return pp.tile([128, 512], F32, tag="ps", name=name)

            # ---- QKV^T projection: psum[col128, s512] = W_chunk.T @ hsT ----
            for ct in range(8):
                for which, wsb in ((0, sb_wq), (1, sb_wk), (2, sb_wv)):
                    ps = ps_tile(f"ps_qkv{which}_{ct}")
                    for kc in range(8):
                        nc.tensor.matmul(
                            ps[:, :], lhsT=wsb[:, kc, ct, :], rhs=sb_hsT[:, kc, :],
                            start=(kc == 0), stop=(kc == 7),
                        )
                    bi = which * 8 + ct
                    if which == 0:
                        dsts = [(sb_qT[:, ct, :], ps[:, :], sb_bqkv[:, bi:bi + 1])]
                    elif which == 1:
                        dsts = [(sb_kT[:, ct, :], ps[:, :], sb_bqkv[:, bi:bi + 1])]
                    else:
                        dsts = [
                            (sb_vta[2 * ct][0:64, :], ps[0:64, :],
                             sb_bqkv[0:64, bi:bi + 1]),
                            (sb_vta[2 * ct + 1][0:64, :], ps[64:128, :],
                             sb_bqkv[64:128, bi:bi + 1]),
                        ]
                    for dst, src_ap, bias_ap in dsts:
                        nc.vector.tensor_copy(dst, src_ap)
                        if not qb0:
                            nc.vector.tensor_scalar_add(dst, dst, bias_ap)

            # ---- per-head attention ----
            for h in range(H):
                hp, hl = divmod(h, 2)
                pof = hl * 64
                qT_h = sb_qT[pof:pof + 64, hp, :]     # [64, 512]
                kT_h = sb_kT[pof:pof + 64, hp, :]

                # prompt scores^T [p64, s512] -> exp -> *mask -> pp_h
                ps_pr = ps_tile(f"ps_pr{h}")
                nc.tensor.matmul(
                    ps_pr[0:64, :], lhsT=sb_pkt[pof:pof + 64, hp, :], rhs=qT_h,
                    start=True, stop=True)
                pp_h = wp.tile([64, 512], BF16, tag="pph", name=f"pp{h}")
                nc.scalar.activation(
                    pp_h[:, :], ps_pr[0:64, :], AF.Exp, scale=SCALE)
                nc.vector.tensor_mul(pp_h[:, :], pp_h[:, :], sb_pmt[:, :])

                # self-term: wself = exp(sum_d qT*kT / 8)  [1, 512]
                tmpqk = wp.tile([64, 512], BF16, tag="tmpqk", name=f"tmpqk{h}")
                nc.vector.tensor_mul(tmpqk[:, :], qT_h, kT_h)
                ps_sw = ps_tile(f"ps_sw{h}")
                nc.tensor.matmul(
                    ps_sw[0:1, :], lhsT=ones_col[0:64, :], rhs=tmpqk[:, :],
                    start=True, stop=True)
                eself = wp.tile([1, 512], BF16, tag="eself", name=f"eself{h}")
                nc.scalar.activation(
                    eself[:, :], ps_sw[0:1, :], AF.Exp, scale=SCALE)
                # wselfB [65, 512] = ones(65) x eself
                ps_wsb = ps_tile(f"ps_wsb{h}")
                nc.tensor.matmul(
                    ps_wsb[0:65, :], lhsT=ones_row[0:1, 0:65],
                    rhs=eself[:, :], start=True, stop=True)

                # text scores^T, exp, mask, AV accumulation
                ps_av = ps_tile(f"ps_av{h}")
                for slot in range(SLOTS):
                    ext = EXT[slot]
                    qslot = sb_qT[pof:pof + 64, hp, slot * 128:slot * 128 + 128]
                    ptile = wp.tile([128, 8, 128], BF16, tag="pt", name=f"pt{h}_{slot}")
                    n_ps = (ext + 3) // 4
                    for half in range(n_ps):
                        j0, j1 = half * 4, min(ext, half * 4 + 4)
                        ps_sc = pp.tile([128, 4, 128], F32, tag="ps",
                                        name=f"ps_sc{h}_{slot}_{half}")
                        for j in range(j0, j1):
                            nc.tensor.matmul(
                                ps_sc[:, j - j0, :],
                                lhsT=sb_tkt[pof:pof + 64, hp, j * 128:j * 128 + 128],
                                rhs=qslot, start=True, stop=True)
                        nc.scalar.activation(
                            ptile[:, j0:j1, :], ps_sc[:, 0:j1 - j0, :],
                            AF.Exp, scale=SCALE)
                    # causal/pad masks on the last two tiles
                    nc.vector.tensor_mul(
                        ptile[:, ext - 2:ext, :], ptile[:, ext - 2:ext, :],
                        sb_mks[:, slot, :, :])
                    # AV: [65, s128] += [pV|1].T @ pP  +  sum_j [tV|1].T @ PT_j
                    sl = slice(slot * 128, slot * 128 + 128)
                    nc.tensor.matmul(
                        ps_av[0:65, sl], lhsT=sb_pva[:, h, :], rhs=pp_h[:, sl],
                        start=True, stop=False)
                    for j in range(ext):
                        nc.tensor.matmul(
                            ps_av[0:65, sl], lhsT=sb_tva[:, h, j, :],
                            rhs=ptile[:, j, :], start=False, stop=(j == ext - 1))

                # finalize head: num = av + vta*wselfB ; attnT = num[:64] * recipB
                t1 = wp.tile([65, 512], F32, tag="t1", name=f"t1_{h}")
                nc.vector.tensor_mul(t1[:, :], sb_vta[h][:, :], ps_wsb[0:65, :])
                num = wp.tile([65, 512], F32, tag="num", name=f"num{h}")
                nc.vector.tensor_add(num[:, :], ps_av[0:65, :], t1[:, :])
                rcp = wp.tile([1, 512], F32, tag="rcp", name=f"rcp{h}")
                nc.vector.reciprocal(rcp[:, :], num[64:65, :])
                ps_rb = ps_tile(f"ps_rb{h}")
                nc.tensor.matmul(
                    ps_rb[0:64, :], lhsT=ones_rowf[0:1, 0:64], rhs=rcp[:, :],
                    start=True, stop=True)
                nc.vector.tensor_mul(
                    sb_attnT[hp][pof:pof + 64, :], num[0:64, :], ps_rb[0:64, :])

            # ---- c_proj: out[s128, e512] = sum_hp attnT_hp.T @ PW_hp ----
            for slot in range(SLOTS):
                for eh in range(2):
                    ps_cp = ps_tile(f"ps_cp{slot}_{eh}")
                    for hp in range(8):
                        nc.tensor.matmul(
                            ps_cp[:, :],
                            lhsT=sb_attnT[hp][:, slot * 128:slot * 128 + 128],
                            rhs=sb_pw[:, hp, eh * 512:eh * 512 + 512],
                            start=(hp == 0), stop=(hp == 7))
                    osb = wp.tile([128, 512], F32, tag="osb", name=f"osb{slot}_{eh}")
                    if pjb0:
                        nc.vector.tensor_copy(osb[:, :], ps_cp[:, :])
                    else:
                        nc.vector.tensor_add(
                            osb[:, :], ps_cp[:, :], sb_pb[:, eh * 512:eh * 512 + 512])
                    nc.sync.dma_start(
                        out=out[slot * 128:slot * 128 + 128,
                                eh * 512:eh * 512 + 512],
                        in_=osb[:, :])
    return nc


def _row_idx(par):
    return np.concatenate(
        [np.arange(blk * 128, blk * 128 + 128) for blk in BLOCKS[par]])


def _bf(a):
    return np.ascontiguousarray(a).astype(BFNP)


def _prep(inputs):
    hs = np.asarray(inputs["hidden_states"], np.float32)
    pK = np.asarray(inputs["promptKey"], np.float32)
    pV = np.asarray(inputs["promptValue"], np.float32)
    tK = np.asarray(inputs["textualKey"], np.float32)
    tV = np.asarray(inputs["textualValue"], np.float32)
    pM = np.asarray(inputs["promptMask"]).astype(np.float32)
    W = np.asarray(inputs["c_attn_w"], np.float32)
    bi = np.asarray(inputs["c_attn_b"], np.float32)
    PW = np.asarray(inputs["c_proj_w"], np.float32)
    Pb = np.asarray(inputs["c_proj_b"], np.float32)

    shared = {
        "wq": _bf(W[:, 0:1024].reshape(8, 128, 8, 128).transpose(1, 0, 2, 3)),
        "wk": _bf(W[:, 1024:2048].reshape(8, 128, 8, 128).transpose(1, 0, 2, 3)),
        "wv": _bf(W[:, 2048:3072].reshape(8, 128, 8, 128).transpose(1, 0, 2, 3)),
        "bqkv": np.ascontiguousarray(bi.reshape(24, 128).T),
        "pw": _bf(PW.reshape(8, 128, 1024).transpose(1, 0, 2)),
        "pb": np.ascontiguousarray(np.broadcast_to(Pb, (128, 1024))),
    }

    # per-parity causal/pad masks
    tl = np.arange(128)[:, None]
    sl = np.arange(128)[None, :]
    mks_by_par = []
    for par in range(2):
        m = np.zeros((128, 4, 2, 128), np.float32)
        for slot in range(4):
            for mm in range(2):
                j = EXT[slot] - 2 + mm
                m[:, slot, mm, :] = (j * 128 + tl) < (BLOCKS[par][slot] * 128 + sl)
        mks_by_par.append(_bf(m))

    in_maps = []
    for c in range(NCORES):
        b, par = divmod(c, 2)
        idx = _row_idx(par)
        hsc = hs[b][idx]                       # [512, 1024]
        m = dict(shared)
        m["hsT"] = _bf(hsc.reshape(512, 8, 128).transpose(2, 1, 0))
        m["tkt"] = _bf(
            tK[b].transpose(0, 2, 1).reshape(8, 2, 64, 1024)
            .transpose(1, 2, 0, 3).reshape(128, 8, 1024))
        tva = tV[b].reshape(16, 8, 128, 64).transpose(2, 0, 1, 3)
        m["tva"] = _bf(np.concatenate(
            [tva, np.ones((128, 16, 8, 1), np.float32)], axis=-1))
        m["pkt"] = _bf(
            pK[b].transpose(0, 2, 1).reshape(8, 2, 64, 64)
            .transpose(1, 2, 0, 3).reshape(128, 8, 64))
        pva = pV[b].transpose(1, 0, 2)
        m["pva"] = _bf(np.concatenate(
            [pva, np.ones((64, 16, 1), np.float32)], axis=-1))
        m["mks"] = mks_by_par[par]
        m["pmt"] = _bf(pM[b, 0][idx].T)
        in_maps.append(m)
    return in_maps


def _get_nc(qb0=True, pjb0=True):
    key = ("nc", qb0, pjb0)
    if key not in _CACHE:
        _CACHE[key] = _build(qb0, pjb0)
    return _CACHE[key]


def _assemble(results):
    full = np.empty((B, S, E), np.float32)
    for c in range(NCORES):
        b, par = divmod(c, 2)
        full[b][_row_idx(par)] = results[c]["out"]
    return full


def _bias_flags(inputs):
    qb0 = not np.any(np.asarray(inputs["c_attn_b"]))
    pjb0 = not np.any(np.asarray(inputs["c_proj_b"]))
    return qb0, pjb0


def kernel(**inputs):
    nc = _get_nc(*_bias_flags(inputs))
    in_maps = _prep(inputs)
    res = bass_utils.run_bass_kernel_spmd(
        nc, in_maps, core_ids=list(range(NCORES)))
    return _assemble(res.results)


def _install_ntff_hook():
    """The agent image's antenv lacks axon_hooks; recreate it so
    run_bass_kernel_spmd(trace=True) can capture NTFF profiles."""
    import sys
    import types
    if "antenv.axon_hooks" in sys.modules:
        return
    import antenv
    from trn_agent_boot.trn_boot import _ntff_profile_via_ctypes
    mod = types.ModuleType("antenv.axon_hooks")
    hook = _ntff_profile_via_ctypes("/opt/axon/libaxon_pjrt.so")
    mod.get_axon_ntff_profile_hook = lambda: hook
    mod.set_axon_ntff_profile_hook = lambda h: None
    sys.modules["antenv.axon_hooks"] = mod
    antenv.axon_hooks = mod
    # zero-egress container: make artifact upload a local no-op
    bass_utils.upload_artifacts = lambda d: str(d)


def kernel_traced(inputs, trace_cores=None, tmpdir=None):
    """For test.py: run with NTFF tracing, return (output, exec_time_ns, res)."""
    _install_ntff_hook()
    nc = _get_nc(*_bias_flags(inputs))
    in_maps = _prep(inputs)
    res = bass_utils.run_bass_kernel_spmd(
        nc, in_maps, core_ids=list(range(NCORES)), trace=True,
        trace_cores=trace_cores or list(range(NCORES)), tmpdir=tmpdir)
    return _assemble(res.results), res.exec_time_ns, res


# revision 14
# speedup vs baseline: 1.3307x; 1.3307x over previous
"""GPT2 non-residual attention on 8 TRN2 NeuronCores (Bass/Tile).

Sharding: rows of (B*S) split 8 ways — core c owns batch b=c//2 and four
interleaved 128-row s-blocks chosen so causal work is balanced across the
two cores of a batch (parity 0 -> blocks {0,3,4,7}, parity 1 -> {1,2,5,6};
both have total causal extent 18 tiles, padded to a uniform static
schedule of EXT=(2,4,6,8) tiles/slot with data-side masks).

All matmuls run in bf16 (rel-err gate is 2e-2; bf16 lands ~1e-3).
Attention uses transposed-score layout: scores^T [t,128s] tiles on PSUM,
exp on ScalarE (scale=1/8 fused, no max pass — scores are provably small),
AV as lhsT=[tV|ones] so the softmax denominator falls out of the matmul,
self-term (q.k diagonal) via ones-matmul partition reduction, per-head
normalization with K=1 broadcast matmuls, c_proj directly from the
transposed attention output.
"""

import numpy as np
import ml_dtypes

import concourse.bass as bass
import concourse.mybir as mybir
import concourse.tile as tile
from concourse import bass_utils

BF16 = mybir.dt.bfloat16
F32 = mybir.dt.float32
AF = mybir.ActivationFunctionType


def _legalize_multi_waits(bir_json):
    """This walrus build encodes at most ONE sem-wait per instruction.
    Split instructions with N>1 waits: hoist N-1 waits onto standalone
    EventSemaphore instructions inserted just before (same engine, same
    program position => identical semantics)."""
    import orjson
    d = orjson.loads(bir_json)
    ctr = [0]
    changed = False
    for fn in d.get("functions", []):
        for blk in fn.get("blocks", []):
            out = []
            for inst in blk.get("instructions", []):
                si = inst.get("sync_info")
                waits = (si or {}).get("on_wait") or []
                if len(waits) > 1:
                    changed = True
                    for w in waits[:-1]:
                        ctr[0] += 1
                        ev = {
                            "engine": inst["engine"],
                            "ins": [],
                            "outs": [],
                            "name": f"WSPLIT-{ctr[0]}",
                            "opcode": "EventSemaphore",
                            "sync_info": {"on_update": [], "on_wait": [w]},
                        }
                        if "debug" in inst:
                            ev["debug"] = inst["debug"]
                        out.append(ev)
                    si["on_wait"] = [waits[-1]]
                out.append(inst)
            blk["instructions"] = out
    return orjson.dumps(d) if changed else bir_json


def _install_wait_legalizer():
    if getattr(bass_utils, "_wait_legalizer_installed", False):
        return
    import concourse.bass2jax as bass2jax
    orig = bass_utils.compile_bir_kernel

    def patched(bir_json, tmpdir, neff_name="file.neff"):
        return orig(_legalize_multi_waits(bir_json), tmpdir, neff_name)

    bass_utils.compile_bir_kernel = patched
    bass2jax.compile_bir_kernel = patched

    bass_utils._wait_legalizer_installed = True


_install_wait_legalizer()

B, S, E = 4, 1024, 1024
H, DH = 16, 64
PL = 64
NCORES = 8
SLOTS = 4
EXT = (2, 4, 6, 8)                      # padded t-tile extents (uniform)
BLOCKS = ((0, 3, 4, 7), (1, 2, 5, 6))   # parity -> s-block ids
SLOT_LO = (0, 0, 1, 1, 2, 2, 3, 3)      # first live slot per t-tile j
SCALE = 0.125                            # 1/sqrt(DH)
BFNP = ml_dtypes.bfloat16

_CACHE: dict = {}


def _build(qb0=True, pjb0=True):
    nc = bass.Bass("TRN2", target_bir_lowering=False, debug=False)

    def din(name, shape, dtype=BF16):
        return nc.dram_tensor(name, list(shape), dtype, kind="ExternalInput").ap()

    hsT = din("hsT", (128, 8, 512))        # (p, kc, s): hs_rows[s, kc*128+p]
    wq = din("wq", (128, 8, 8, 128))       # (p, kc, ct, c)
    wk = din("wk", (128, 8, 8, 128))
    wv = din("wv", (128, 8, 8, 128))
    bqkv = din("bqkv", (128, 24), F32)     # bias^T col-tiles: q 0-7, k 8-15, v 16-23
    tkt = din("tkt", (128, 8, 1024))       # ((h%2)*64+d, hp, t)
    tva = din("tva", (128, 16, 8, 65))     # (t_loc, h, j, d|ones)
    pkt = din("pkt", (128, 8, 64))         # ((h%2)*64+d, hp, p)
    pva = din("pva", (64, 16, 65))         # (p, h, d|ones)
    mks = din("mks", (128, 4, 2, 128))     # (t_loc, slot, m, s_loc)
    pmt = din("pmt", (64, 512))            # prompt mask^T (p, s)
    pw = din("pw", (128, 8, 1024))         # (p, fc, e)
    pb = din("pb", (128, 1024), F32)       # c_proj bias broadcast over partitions
    out = nc.dram_tensor("out", [512, 1024], F32, kind="ExternalOutput").ap()

    with tile.TileContext(nc) as tc:
        with (
            tc.tile_pool(name="const", bufs=1) as cp,
            tc.tile_pool(name="work", bufs=3) as wp,
            tc.tile_pool(name="psum", bufs=8, space="PSUM") as pp,
        ):
            def ctile(name, shape, dtype=BF16):
                t = cp.tile(list(shape), dtype, tag=name, name=name)
                return t

            sb_hsT = ctile("sb_hsT", (128, 8, 512))
            sb_wq = ctile("sb_wq", (128, 8, 8, 128))
            sb_wk = ctile("sb_wk", (128, 8, 8, 128))
            sb_wv = ctile("sb_wv", (128, 8, 8, 128))
            sb_bqkv = ctile("sb_bqkv", (128, 24), F32)
            sb_tkt = ctile("sb_tkt", (128, 8, 1024))
            sb_tva = ctile("sb_tva", (128, 16, 8, 65))
            sb_pkt = ctile("sb_pkt", (128, 8, 64))
            sb_pva = ctile("sb_pva", (64, 16, 65))
            sb_mks = ctile("sb_mks", (128, 4, 2, 128))
            sb_pmt = ctile("sb_pmt", (64, 512))
            sb_pw = ctile("sb_pw", (128, 8, 1024))
            sb_pb = ctile("sb_pb", (128, 1024), F32)

            sb_qT = ctile("sb_qT", (128, 8, 512))
            sb_kT = ctile("sb_kT", (128, 8, 512))
            sb_vta = [ctile(f"sb_vta{h}", (64, 512)) for h in range(H)]
            sb_attnT = [ctile(f"sb_attnT{hp}", (128, 512)) for hp in range(8)]
            ones_col = ctile("ones_col", (128, 1))          # bf16 ones (lhsT self)
            ones_row = ctile("ones_row", (1, 128))          # bf16 ones (K=1 bcast)

            dmas = [
                (hsT, sb_hsT), (wq, sb_wq), (wk, sb_wk), (wv, sb_wv),
                (tkt, sb_tkt), (tva, sb_tva), (pkt, sb_pkt),
                (pva, sb_pva), (mks, sb_mks), (pmt, sb_pmt), (pw, sb_pw),
            ]
            if not qb0:
                dmas.append((bqkv, sb_bqkv))
            if not pjb0:
                dmas.append((pb, sb_pb))
            for ap_in, sb in dmas:
                nc.sync.dma_start(out=sb[:], in_=ap_in[:])

            nc.vector.memset(ones_col[:], 1.0)
            nc.vector.memset(ones_row[:], 1.0)

            def ps_tile(name):
                return pp.tile([128, 512], F32, tag="ps", name=name)

            # ---- QKV^T projection: psum[col128, s512] = W_chunk.T @ hsT ----
            for ct in range(8):
                for which, wsb in ((0, sb_wq), (1, sb_wk), (2, sb_wv)):
                    ps = ps_tile(f"ps_qkv{which}_{ct}")
                    for kc in range(8):
                        nc.tensor.matmul(
                            ps[:, :], lhsT=wsb[:, kc, ct, :], rhs=sb_hsT[:, kc, :],
                            start=(kc == 0), stop=(kc == 7),
                        )
                    bi = which * 8 + ct
                    if which == 0:
                        dsts = [(sb_qT[:, ct, :], ps[:, :], sb_bqkv[:, bi:bi + 1])]
                    elif which == 1:
                        dsts = [(sb_kT[:, ct, :], ps[:, :], sb_bqkv[:, bi:bi + 1])]
                    else:
                        dsts = [
                            (sb_vta[2 * ct][:, :], ps[0:64, :],
                             sb_bqkv[0:64, bi:bi + 1]),
                            (sb_vta[2 * ct + 1][:, :], ps[64:128, :],
                             sb_bqkv[64:128, bi:bi + 1]),
                        ]
                    for dst, src_ap, bias_ap in dsts:
                        nc.vector.tensor_copy(dst, src_ap)
                        if not qb0:
                            nc.vector.tensor_scalar_add(dst, dst, bias_ap)

            # ---- per-head attention ----
            for h in range(H):
                hp, hl = divmod(h, 2)
                pof = hl * 64
                qT_h = sb_qT[pof:pof + 64, hp, :]     # [64, 512]
                kT_h = sb_kT[pof:pof + 64, hp, :]

                # prompt scores^T [p64, s512] -> exp -> *mask -> pp_h
                ps_pr = ps_tile(f"ps_pr{h}")
                nc.tensor.matmul(
                    ps_pr[0:64, :], lhsT=sb_pkt[pof:pof + 64, hp, :], rhs=qT_h,
                    start=True, stop=True)
                pp_h = wp.tile([64, 512], BF16, tag="pph", name=f"pp{h}")
                nc.scalar.activation(
                    pp_h[:, :], ps_pr[0:64, :], AF.Exp, scale=SCALE)
                nc.vector.tensor_mul(pp_h[:, :], pp_h[:, :], sb_pmt[:, :])

                # self-term: wself = exp(sum_d qT*kT / 8)  [1, 512]
                tmpqk = wp.tile([64, 512], BF16, tag="tmpqk", name=f"tmpqk{h}")
                nc.vector.tensor_mul(tmpqk[:, :], qT_h, kT_h)
                ps_sw = ps_tile(f"ps_sw{h}")
                nc.tensor.matmul(
                    ps_sw[0:1, :], lhsT=ones_col[0:64, :], rhs=tmpqk[:, :],
                    start=True, stop=True)
                eself = wp.tile([1, 512], BF16, tag="eself", name=f"eself{h}")
                nc.scalar.activation(
                    eself[:, :], ps_sw[0:1, :], AF.Exp, scale=SCALE)
                # eselfB [64, 512] = ones(64) x eself  (for the v*wself term)
                es_ps = ps_tile(f"ps_es{h}")
                nc.tensor.matmul(
                    es_ps[0:64, :], lhsT=ones_row[0:1, 0:64],
                    rhs=eself[:, :], start=True, stop=True)

                # text scores^T by t-tile across all live slots, exp, mask,
                # then AV accumulation (one matmul per t-tile)
                ps_av = ps_tile(f"ps_av{h}")
                pt_all = wp.tile([128, 8, 4, 128], BF16, tag="pt", bufs=2,
                                 name=f"pt{h}")
                for j in range(8):
                    lo = SLOT_LO[j]
                    ps_sc = pp.tile([128, 4, 128], F32, tag="ps",
                                    name=f"ps_sc{h}_{j}")
                    nc.tensor.matmul(
                        ps_sc[:, 0:4 - lo, :],
                        lhsT=sb_tkt[pof:pof + 64, hp, j * 128:j * 128 + 128],
                        rhs=sb_qT[pof:pof + 64, hp, lo * 128:512],
                        start=True, stop=True)
                    nc.scalar.activation(
                        pt_all[:, j, lo:4, :], ps_sc[:, 0:4 - lo, :],
                        AF.Exp, scale=SCALE)
                for slot in range(SLOTS):
                    ext = EXT[slot]
                    nc.vector.tensor_mul(
                        pt_all[:, ext - 2:ext, slot, :],
                        pt_all[:, ext - 2:ext, slot, :],
                        sb_mks[:, slot, :, :])
                nc.tensor.matmul(
                    ps_av[0:65, :], lhsT=sb_pva[:, h, :], rhs=pp_h[:, :],
                    start=True, stop=False, skip_group_check=True)
                for j in range(8):
                    lo = SLOT_LO[j]
                    nc.tensor.matmul(
                        ps_av[0:65, lo * 128:512], lhsT=sb_tva[:, h, j, :],
                        rhs=pt_all[:, j, lo:4, :],
                        start=False, stop=(j == 7), skip_group_check=True)

                # denom = av row64 + self weight, broadcast via K=1 matmul,
                # then fast approx reciprocal (single custom-DVE op)
                den1 = wp.tile([1, 512], BF16, tag="den1", name=f"den1_{h}")
                nc.vector.tensor_add(den1[:, :], ps_av[64:65, :], eself[:, :])
                den_ps = ps_tile(f"ps_den{h}")
                nc.tensor.matmul(
                    den_ps[0:64, :], lhsT=ones_row[0:1, 0:64], rhs=den1[:, :],
                    start=True, stop=True)
                # 1/d as exp(-ln d) on the (underused) scalar engine
                lnb = wp.tile([64, 512], F32, tag="lnb", name=f"lnb{h}")
                nc.scalar.activation(lnb[:, :], den_ps[0:64, :], AF.Ln)
                rcpB = wp.tile([64, 512], F32, tag="rcpB", name=f"rcpB{h}")
                nc.scalar.activation(rcpB[:, :], lnb[:, :], AF.Exp, scale=-1.0)
                # attnT = (av[:64] + vT*eselfB) * recipB
                m1 = wp.tile([64, 512], F32, tag="m1", name=f"m1_{h}")
                nc.vector.tensor_mul(m1[:, :], sb_vta[h][:, :], es_ps[0:64, :])
                nc.vector.tensor_add(m1[:, :], ps_av[0:64, :], m1[:, :])
                nc.vector.tensor_mul(
                    sb_attnT[hp][pof:pof + 64, :], m1[:, :], rcpB[0:64, :])

            # ---- c_proj: out[s128, e512] = sum_hp attnT_hp.T @ PW_hp ----
            for slot in range(SLOTS):
                for eh in range(2):
                    ps_cp = ps_tile(f"ps_cp{slot}_{eh}")
                    for hp in range(8):
                        nc.tensor.matmul(
                            ps_cp[:, :],
                            lhsT=sb_attnT[hp][:, slot * 128:slot * 128 + 128],
                            rhs=sb_pw[:, hp, eh * 512:eh * 512 + 512],
                            start=(hp == 0), stop=(hp == 7))
                    osb = wp.tile([128, 512], F32, tag="osb", name=f"osb{slot}_{eh}")
                    if pjb0:
                        nc.vector.tensor_copy(osb[:, :], ps_cp[:, :])
                    else:
                        nc.vector.tensor_add(
                            osb[:, :], ps_cp[:, :], sb_pb[:, eh * 512:eh * 512 + 512])
                    nc.sync.dma_start(
                        out=out[slot * 128:slot * 128 + 128,
                                eh * 512:eh * 512 + 512],
                        in_=osb[:, :])
    return nc


def _row_idx(par):
    return np.concatenate(
        [np.arange(blk * 128, blk * 128 + 128) for blk in BLOCKS[par]])


def _bf(a):
    return np.ascontiguousarray(a).astype(BFNP)


def _prep(inputs):
    hs = np.asarray(inputs["hidden_states"], np.float32)
    pK = np.asarray(inputs["promptKey"], np.float32)
    pV = np.asarray(inputs["promptValue"], np.float32)
    tK = np.asarray(inputs["textualKey"], np.float32)
    tV = np.asarray(inputs["textualValue"], np.float32)
    pM = np.asarray(inputs["promptMask"]).astype(np.float32)
    W = np.asarray(inputs["c_attn_w"], np.float32)
    bi = np.asarray(inputs["c_attn_b"], np.float32)
    PW = np.asarray(inputs["c_proj_w"], np.float32)
    Pb = np.asarray(inputs["c_proj_b"], np.float32)

    shared = {
        "wq": _bf(W[:, 0:1024].reshape(8, 128, 8, 128).transpose(1, 0, 2, 3)),
        "wk": _bf(W[:, 1024:2048].reshape(8, 128, 8, 128).transpose(1, 0, 2, 3)),
        "wv": _bf(W[:, 2048:3072].reshape(8, 128, 8, 128).transpose(1, 0, 2, 3)),
        "bqkv": np.ascontiguousarray(bi.reshape(24, 128).T),
        "pw": _bf(PW.reshape(8, 128, 1024).transpose(1, 0, 2)),
        "pb": np.ascontiguousarray(np.broadcast_to(Pb, (128, 1024))),
    }

    # per-parity causal/pad masks
    tl = np.arange(128)[:, None]
    sl = np.arange(128)[None, :]
    mks_by_par = []
    for par in range(2):
        m = np.zeros((128, 4, 2, 128), np.float32)
        for slot in range(4):
            for mm in range(2):
                j = EXT[slot] - 2 + mm
                m[:, slot, mm, :] = (j * 128 + tl) < (BLOCKS[par][slot] * 128 + sl)
        mks_by_par.append(_bf(m))

    in_maps = []
    for c in range(NCORES):
        b, par = divmod(c, 2)
        idx = _row_idx(par)
        hsc = hs[b][idx]                       # [512, 1024]
        m = dict(shared)
        m["hsT"] = _bf(hsc.reshape(512, 8, 128).transpose(2, 1, 0))
        m["tkt"] = _bf(
            tK[b].transpose(0, 2, 1).reshape(8, 2, 64, 1024)
            .transpose(1, 2, 0, 3).reshape(128, 8, 1024))
        tva = tV[b].reshape(16, 8, 128, 64).transpose(2, 0, 1, 3)
        m["tva"] = _bf(np.concatenate(
            [tva, np.ones((128, 16, 8, 1), np.float32)], axis=-1))
        m["pkt"] = _bf(
            pK[b].transpose(0, 2, 1).reshape(8, 2, 64, 64)
            .transpose(1, 2, 0, 3).reshape(128, 8, 64))
        pva = pV[b].transpose(1, 0, 2)
        m["pva"] = _bf(np.concatenate(
            [pva, np.ones((64, 16, 1), np.float32)], axis=-1))
        m["mks"] = mks_by_par[par]
        m["pmt"] = _bf(pM[b, 0][idx].T)
        in_maps.append(m)
    return in_maps


def _get_nc(qb0=True, pjb0=True):
    key = ("nc", qb0, pjb0)
    if key not in _CACHE:
        _CACHE[key] = _build(qb0, pjb0)
    return _CACHE[key]


def _assemble(results):
    full = np.empty((B, S, E), np.float32)
    for c in range(NCORES):
        b, par = divmod(c, 2)
        full[b][_row_idx(par)] = results[c]["out"]
    return full


def _bias_flags(inputs):
    qb0 = not np.any(np.asarray(inputs["c_attn_b"]))
    pjb0 = not np.any(np.asarray(inputs["c_proj_b"]))
    return qb0, pjb0


def kernel(**inputs):
    nc = _get_nc(*_bias_flags(inputs))
    in_maps = _prep(inputs)
    res = bass_utils.run_bass_kernel_spmd(
        nc, in_maps, core_ids=list(range(NCORES)))
    return _assemble(res.results)


def _install_ntff_hook():
    """The agent image's antenv lacks axon_hooks; recreate it so
    run_bass_kernel_spmd(trace=True) can capture NTFF profiles."""
    import sys
    import types
    if "antenv.axon_hooks" in sys.modules:
        return
    import antenv
    from trn_agent_boot.trn_boot import _ntff_profile_via_ctypes
    mod = types.ModuleType("antenv.axon_hooks")
    hook = _ntff_profile_via_ctypes("/opt/axon/libaxon_pjrt.so")
    mod.get_axon_ntff_profile_hook = lambda: hook
    mod.set_axon_ntff_profile_hook = lambda h: None
    sys.modules["antenv.axon_hooks"] = mod
    antenv.axon_hooks = mod
    # zero-egress container: make artifact upload a local no-op
    bass_utils.upload_artifacts = lambda d: str(d)


def kernel_traced(inputs, trace_cores=None, tmpdir=None):
    """For test.py: run with NTFF tracing, return (output, exec_time_ns, res)."""
    _install_ntff_hook()
    nc = _get_nc(*_bias_flags(inputs))
    in_maps = _prep(inputs)
    res = bass_utils.run_bass_kernel_spmd(
        nc, in_maps, core_ids=list(range(NCORES)), trace=True,
        trace_cores=trace_cores or list(range(NCORES)), tmpdir=tmpdir)
    return _assemble(res.results), res.exec_time_ns, res
